# revision 5
# baseline (speedup 1.0000x reference)
# kernel2.py — MoE (E=16, top-4) Trainium2 Bass kernel, expert-parallel over 8 cores.
#
# v2 changes vs baseline:
#   - bf16 expert weights + activations (halves the dominant weight DMA stream)
#   - matmul-based dispatch compaction (one-hot @ token-ids) instead of
#     48 tiny indirect-DMA scatters + DRAM round trip
#   - combine weights fused into the x-row gather via xcomb [T, H+NSLOT] bf16
#   - phase-2 loads on the Act queue so the SP queue streams weights from t=0
#   - bf16 output accumulator + bf16 ReduceScatters
import numpy as np

H = 1024
F = 4096
E = 16
TOPK = 4
T = 2048
NCORES = 8
TSH = T // NCORES          # 256 router tokens per core
NEG = -3.0e38
MARGIN = 16                # slack over host-computed counts (host/device drift)

_CACHE = {}


# ---------------------------------------------------------------------------
# Host-side planning: counts -> slot capacities + (expert, lo) assignment
# ---------------------------------------------------------------------------

def _host_counts(inputs):
    x = np.asarray(inputs["x"], np.float32).reshape(T, H)
    h = np.maximum(x @ np.asarray(inputs["Wr1"], np.float32)
                   + np.asarray(inputs["br1"], np.float32), 0.0)
    lg = h @ np.asarray(inputs["Wr2"], np.float32) + np.asarray(inputs["br2"], np.float32)
    order = np.argsort(-lg, axis=1, kind="stable")[:, :TOPK]
    counts = np.zeros(E, np.int64)
    for e in range(E):
        counts[e] = (order == e).sum()
    return counts


def _solve_pack(needs, caps):
    """Exact DFS: assign each expert a multiset of slots (one per piece) with
    slot-sum >= need. Returns per-expert slot-size lists (aligned with the
    order of `needs`) or None."""
    order = sorted(range(len(needs)), key=lambda i: -needs[i])
    sizes = sorted(set(caps), reverse=True)
    avail0 = tuple(sum(8 for c in caps if c == s) for s in sizes)

    def dfs(i, avail):
        if i == len(order):
            return {}
        need = needs[order[i]]
        rem_need = sum(needs[order[j]] for j in range(i, len(order)))
        if sum(s * c for s, c in zip(sizes, avail)) < rem_need:
            return None
        options = []

        def gen(j, chosen, ssum):
            if ssum >= need:
                options.append((len(chosen), ssum - need, tuple(chosen)))
                return
            if j == len(sizes) or len(chosen) >= 8:
                return
            taken_j = sum(1 for c in chosen if c == j)
            if avail[j] - taken_j > 0:
                gen(j, chosen + [j], ssum + sizes[j])
            gen(j + 1, chosen, ssum)

        gen(0, [], 0)
        options.sort()
        for _, _, chosen in options[:8]:
            av2 = list(avail)
            for c in chosen:
                av2[c] -= 1
            if min(av2) < 0:
                continue
            rest = dfs(i + 1, tuple(av2))
            if rest is not None:
                rest[order[i]] = [sizes[c] for c in chosen]
                return rest
        return None

    return dfs(0, avail0)


def _plan(inputs):
    counts = _host_counts(inputs)
    needs = [int(c) + MARGIN for c in counts]
    cands = []
    for n in (3, 4, 5):
        def rec(pref, lo):
            if len(pref) == n:
                cands.append(tuple(pref))
                return
            for a in range(lo, 0, -128):
                rec(pref + [a], a)
        rec([], 640)   # caps > 640 would exceed the PSUM bank budget in mm2
    # prefer fewer slots (less weight DMA), then smaller total capacity
    cands = [c for c in cands if sum(c) * 8 >= sum(needs)]
    cands.sort(key=lambda c: (len(c), sum(c), max(c)))
    for caps in cands:
        sol = _solve_pack(needs, list(caps))
        if sol is None:
            continue
        # build per-(position, core) assignment
        by_size = {}
        for e, sls in sol.items():
            lo = 0
            for s in sorted(sls, reverse=True):
                by_size.setdefault(s, []).append((e, lo))
                lo += s
        asg = []
        used = {s: 0 for s in by_size}
        for a in caps:
            pos = []
            for r in range(NCORES):
                lst = by_size.get(a, [])
                i = used.get(a, 0)
                if i < len(lst):
                    pos.append(lst[i])
                    used[a] = i + 1
                else:
                    pos.append((0, T + 4096))   # empty slot
            asg.append(pos)
        return list(caps), asg
    raise RuntimeError(f"no feasible slot packing for counts {counts}")


# ---------------------------------------------------------------------------
# Device program
# ---------------------------------------------------------------------------

def _build(caps):
    import concourse.bass as bass
    import concourse.mybir as mybir
    import concourse.tile as tile
    from concourse import bacc
    from concourse.masks import make_identity

    dt = mybir.dt
    BF = dt.bfloat16
    FP16 = dt.float16
    F32R = dt.float32r
    f32 = dt.float32
    i32 = dt.int32
    Alu = mybir.AluOpType
    Act = mybir.ActivationFunctionType
    NSLOT = len(caps)
    CMAX = max(caps)
    HC = H + NSLOT             # xcomb row width
    DUMP = T
    FO = 512                   # mm1 f-block per w1 tile
    HH = H // 2                # mm2 output half width

    nc = bacc.Bacc(None, target_bir_lowering=False, debug=False, num_devices=NCORES)

    # ---------------- I/O ----------------
    xcomb = nc.dram_tensor("xcomb", [T, HC], BF, kind="ExternalInput")
    xsh = nc.dram_tensor("xsh", [TSH, H], f32, kind="ExternalInput")
    Wr1 = nc.dram_tensor("Wr1", [H, H], f32, kind="ExternalInput")
    br1 = nc.dram_tensor("br1", [H], f32, kind="ExternalInput")
    Wr2 = nc.dram_tensor("Wr2", [H, E], f32, kind="ExternalInput")
    br2 = nc.dram_tensor("br2", [E], f32, kind="ExternalInput")
    W1loc = nc.dram_tensor("W1loc", [NSLOT, H, F], BF, kind="ExternalInput")
    b1loc = nc.dram_tensor("b1loc", [NSLOT, F], f32, kind="ExternalInput")
    W2loc = nc.dram_tensor("W2loc", [NSLOT, F, H], BF, kind="ExternalInput")
    b2loc = nc.dram_tensor("b2loc", [NSLOT, H], BF, kind="ExternalInput")
    ohloc = nc.dram_tensor("ohloc", [NSLOT, E], f32, kind="ExternalInput")
    slotlo = nc.dram_tensor("slotlo", [NSLOT], f32, kind="ExternalInput")
    out_sh = nc.dram_tensor("out_sh", [TSH, H], f32, kind="ExternalOutput")

    # ---------------- constants (inline in NEFF) ----------------
    u128 = nc.inline_tensor(np.triu(np.ones((128, 128), np.float32), 1), "u128")
    u16 = nc.inline_tensor(np.triu(np.ones((16, 16), np.float32), 1), "u16")
    ones128 = nc.inline_tensor(np.ones((128, 1), np.float32), "ones128")
    # ids16[p, c] = c*128 + p + 1 (fp16 exact for <= 2048)
    ids_np = (np.arange(16)[None, :] * 128 + np.arange(128)[:, None] + 1).astype(np.float16)
    ids16 = nc.inline_tensor(ids_np, "ids16")
    iota_np = np.broadcast_to(np.arange(CMAX, dtype=np.float16)[None, :], (128, CMAX)).copy()
    iota16 = nc.inline_tensor(iota_np, "iota16")

    # ---------------- internal DRAM ----------------
    agin = nc.dram_tensor("agin", [TSH, E], f32)
    call = nc.dram_tensor("call", [T, E], f32, addr_space="Shared")
    outp = nc.dram_tensor("outp", [T + 1, H], BF)
    rsout = nc.dram_tensor("rsout", [TSH, H], BF)

    RG = [list(range(NCORES))]

    with tile.TileContext(nc) as tc:
        with (
            tc.tile_pool(name="const", bufs=1) as constp,
            tc.tile_pool(name="persist", bufs=1) as persist,
        ):
            ident = constp.tile([128, 128], f32)
            make_identity(nc, ident)
            identb = constp.tile([128, 128], BF)
            nc.vector.tensor_copy(identb[:], ident[:])
            u128_sb = constp.tile_from(u128.ap())
            u16_sb = constp.tile_from(u16.ap())
            ones128_sb = constp.tile_from(ones128.ap())
            ids16_sb = constp.tile_from(ids16.ap())
            iota16_sb = constp.tile_from(iota16.ap())
            onesmm_f32 = constp.tile([1, 128], f32)
            nc.vector.memset(onesmm_f32[:], 1.0)
            onesmm_sb = constp.tile([1, 128], BF)
            nc.vector.tensor_copy(onesmm_sb[:], onesmm_f32[:])
            zero_sb = constp.tile([128, HH], BF)
            nc.vector.memset(zero_sb[:], 0.0)

            # ====== phase 1: router on this core's 256-token shard (f32r) ======
            with (
                tc.tile_pool(name="rweights", bufs=1) as rw,
                tc.tile_pool(name="rtmp", bufs=3) as rt,
                tc.tile_pool(name="rpsum", bufs=2, space="PSUM") as rp,
                tc.tile_pool(name="rtpsum", bufs=2, space="PSUM") as rtp,
            ):
                xt_sh = rw.tile([128, H // 128, TSH], f32)
                for t2 in range(TSH // 128):
                    xs = rt.tile([128, H], f32, tag="xs")
                    nc.sync.dma_start(xs[:], xsh[t2 * 128:(t2 + 1) * 128, :])
                    for hcc in range(H // 128):
                        tp = rtp.tile([128, 128], f32, tag="tp")
                        nc.tensor.transpose(tp[:], xs[:, hcc * 128:(hcc + 1) * 128], ident[:])
                        nc.any.tensor_copy(xt_sh[:, hcc, t2 * 128:(t2 + 1) * 128], tp[:])

                wr1_sb = rw.tile([128, H // 128, H], f32)
                for oh in range(2):
                    nc.sync.dma_start(
                        wr1_sb[:, :, oh * (H // 2):(oh + 1) * (H // 2)],
                        Wr1.ap()[:, oh * (H // 2):(oh + 1) * (H // 2)]
                        .rearrange("(c p) o -> p c o", p=128))
                wr2_sb = rw.tile([128, H // 128, E], f32)
                nc.sync.dma_start(wr2_sb[:], Wr2.ap().rearrange("(c p) e -> p c e", p=128))
                br1_sb = rw.tile([128, H // 128], f32)
                nc.sync.dma_start(br1_sb[:], br1.ap().rearrange("(c p) -> p c", p=128))
                br2_rep = rw.tile([128, E], f32)
                nc.sync.dma_start(
                    br2_rep[:],
                    br2.ap().rearrange("(o e) -> o e", o=1).to_broadcast([128, E]))

                r1t = rw.tile([128, H // 128, TSH], f32)
                for ho in range(H // 128):
                    p1 = rp.tile([128, TSH], f32, tag="p1")
                    for hc in range(H // 128):
                        nc.tensor.matmul(
                            p1[:], wr1_sb[:, hc, ho * 128:(ho + 1) * 128], xt_sh[:, hc, :],
                            start=(hc == 0), stop=(hc == H // 128 - 1))
                    nc.scalar.activation(r1t[:, ho, :], p1[:], Act.Relu,
                                         bias=br1_sb[:, ho:ho + 1])

                for t2 in range(TSH // 128):
                    p2 = rp.tile([128, E], f32, tag="p2")
                    for hc in range(H // 128):
                        nc.tensor.matmul(
                            p2[:], r1t[:, hc, t2 * 128:(t2 + 1) * 128], wr2_sb[:, hc, :],
                            start=(hc == 0), stop=(hc == H // 128 - 1))
                    lg = rt.tile([128, E], f32, tag="lg")
                    nc.vector.tensor_tensor(lg[:], p2[:], br2_rep[:], op=Alu.add)
                    mx8 = rt.tile([128, 8], f32, tag="mx8")
                    nc.vector.max(mx8[:], lg[:])
                    mx4 = rt.tile([128, 8], f32, tag="mx4")
                    nc.vector.memset(mx4[:], NEG)
                    nc.vector.tensor_copy(mx4[:, 0:TOPK], mx8[:, 0:TOPK])
                    zap = rt.tile([128, E], f32, tag="zap")
                    nc.vector.match_replace(zap[:], in_to_replace=mx4[:], in_values=lg[:],
                                            imm_value=NEG)
                    mask = rt.tile([128, E], f32, tag="mask")
                    nc.vector.tensor_tensor(mask[:], lg[:], zap[:], op=Alu.not_equal)
                    negmax = rt.tile([128, 1], f32, tag="negmax")
                    nc.vector.tensor_scalar_mul(negmax[:], mx8[:, 0:1], -1.0)
                    ex = rt.tile([128, E], f32, tag="ex")
                    nc.scalar.activation(ex[:], lg[:], Act.Exp, bias=negmax[:])
                    nc.vector.tensor_tensor(ex[:], ex[:], mask[:], op=Alu.mult)
                    den = rt.tile([128, 1], f32, tag="den")
                    nc.vector.reduce_sum(den[:], ex[:], axis=mybir.AxisListType.X)
                    rcp = rt.tile([128, 1], f32, tag="rcp")
                    nc.vector.reciprocal(rcp[:], den[:])
                    csh = rt.tile([128, E], f32, tag="csh")
                    nc.vector.tensor_scalar(csh[:], ex[:], rcp[:], None, op0=Alu.mult)
                    nc.scalar.dma_start(agin[t2 * 128:(t2 + 1) * 128, :], csh[:])

            nc.gpsimd.collective_compute(
                "AllGather", Alu.bypass, replica_groups=RG,
                ins=[agin.ap().opt()], outs=[call.ap().opt()])


            # ====== phase 2: dispatch for the NSLOT local slots ======
            idx_sb = []
            with (
                tc.tile_pool(name="dsb", bufs=2) as dsb,
                tc.tile_pool(name="dM", bufs=2) as dMp,
                tc.tile_pool(name="dps", bufs=2, space="PSUM") as dps,
                tc.tile_pool(name="dqs", bufs=1, space="PSUM") as dqs,
                tc.tile_pool(name="dts", bufs=1, space="PSUM") as dts,
            ):
                cf = persist.tile([128, T // 128, E], f32, tag="cfall")
                nc.scalar.dma_start(cf[:], call.ap().rearrange("(c p) e -> p c e", p=128))
                ohrep = dsb.tile([128, NSLOT, E], f32, tag="ohrep")
                nc.scalar.dma_start(
                    ohrep[:],
                    ohloc.ap().rearrange("(o l) e -> o l e", o=1).to_broadcast([128, NSLOT, E]))
                lo_rep = dsb.tile([128, NSLOT], f32, tag="lo_rep")
                nc.scalar.dma_start(
                    lo_rep[:],
                    slotlo.ap().rearrange("(o l) -> o l", o=1).to_broadcast([128, NSLOT]))

                c2sb = persist.tile([128, T // 128, NSLOT], f32)
                for k in range(NSLOT):
                    idx_sb.append(persist.tile([128, CMAX // 128], i32,
                                               tag=f"idx{k}", name=f"idx{k}"))

                for k in range(NSLOT):
                    A = caps[k]
                    NB = A // 128
                    msk = dsb.tile([128, T // 128, E], f32, tag="msk")
                    nc.vector.tensor_tensor(
                        msk[:], cf[:],
                        ohrep[:, k:k + 1, :].to_broadcast([128, T // 128, E]),
                        op=Alu.mult)
                    ce = dsb.tile([128, T // 128], f32, tag="ce")
                    nc.vector.reduce_sum(ce[:], msk[:], axis=mybir.AxisListType.X)
                    nc.vector.tensor_copy(c2sb[:, :, k], ce[:])
                    m = dsb.tile([128, T // 128], f32, tag="m")
                    nc.vector.tensor_scalar(m[:], ce[:], 0.0, None, op0=Alu.not_equal)

                    # exclusive cumsum over global token order (partition-inner)
                    csp = dps.tile([16, 1], f32, tag="csp")
                    nc.tensor.matmul(csp[:], m[:], ones128_sb[:], start=True, stop=True)
                    cs_sb = dsb.tile([16, 1], f32, tag="cs_sb")
                    nc.any.tensor_copy(cs_sb[:], csp[:])
                    csrep = dsb.tile([16, 128], f32, tag="csrep")
                    nc.vector.tensor_copy(csrep[:], cs_sb[:].to_broadcast([16, 128]))
                    posp = dps.tile([128, T // 128], f32, tag="posp")
                    nc.tensor.matmul(posp[:], u128_sb[:], m[:], start=True, stop=False)
                    nc.tensor.matmul(posp[:], csrep[:], u16_sb[:], start=False, stop=True)

                    # gate to [lo, lo+A): tpos = pos - lo; m' = m*(tpos>=0)*(tpos<A)
                    tpos = dsb.tile([128, T // 128], f32, tag="tpos")
                    nc.vector.tensor_scalar(tpos[:], posp[:], lo_rep[:, k:k + 1], None,
                                            op0=Alu.subtract)
                    g1 = dsb.tile([128, T // 128], f32, tag="g1")
                    nc.vector.tensor_scalar(g1[:], tpos[:], 0.0, None, op0=Alu.is_ge)
                    g2 = dsb.tile([128, T // 128], f32, tag="g2")
                    nc.vector.tensor_scalar(g2[:], tpos[:], float(A), None, op0=Alu.is_lt)
                    nc.vector.tensor_tensor(m[:], m[:], g1[:], op=Alu.mult)
                    nc.vector.tensor_tensor(m[:], m[:], g2[:], op=Alu.mult)

                    # offsets: O = A + m*(tpos - A)   (unselected -> dump pos A)
                    of = dsb.tile([128, T // 128], f32, tag="of")
                    nc.vector.tensor_scalar(of[:], tpos[:], float(A), None, op0=Alu.subtract)
                    nc.vector.tensor_tensor(of[:], of[:], m[:], op=Alu.mult)
                    nc.vector.tensor_scalar(of[:], of[:], float(A), None, op0=Alu.add)
                    of16 = dsb.tile([128, T // 128], FP16, tag="of16")
                    nc.vector.tensor_copy(of16[:], of[:])

                    # compacted id list via one-hot matmul:
                    # idq[0, q] = sum_t (id_t+1) * 1[of_t == q]
                    nqb = (A + 511) // 512
                    idqs = [dqs.tile([1, 512], f32, tag=f"idq{qb}",
                                     name=f"idq{qb}") for qb in range(nqb)]
                    for c in range(T // 128):
                        Mh = dMp.tile([128, CMAX], FP16, tag="Mh")
                        nc.vector.tensor_tensor(
                            Mh[:, 0:A],
                            of16[:, c:c + 1].to_broadcast([128, A]),
                            iota16_sb[:, 0:A], op=Alu.is_equal)
                        for qb in range(nqb):
                            q0 = qb * 512
                            q1 = min(q0 + 512, A)
                            nc.tensor.matmul(
                                idqs[qb][:, 0:q1 - q0], ids16_sb[:, c:c + 1],
                                Mh[:, q0:q1],
                                start=(c == 0), stop=(c == T // 128 - 1))
                    idrow = dsb.tile([1, CMAX], f32, tag="idrow")
                    for qb in range(nqb):
                        q0 = qb * 512
                        q1 = min(q0 + 512, A)
                        nc.any.tensor_copy(idrow[0:1, q0:q1], idqs[qb][0:1, 0:q1 - q0])
                    idt = dts.tile([128, CMAX // 128], f32, tag="idt")
                    for j in range(NB):
                        nc.tensor.transpose(idt[:, j:j + 1],
                                            idrow[0:1, j * 128:(j + 1) * 128],
                                            ident[0:1, 0:1])
                    # idx = (v == 0) ? DUMP : v - 1
                    vz = dsb.tile([128, CMAX // 128], f32, tag="vz")
                    nc.vector.tensor_scalar(vz[:, 0:NB], idt[:, 0:NB], 0.0, None,
                                            op0=Alu.is_equal)
                    nc.vector.tensor_scalar_mul(vz[:, 0:NB], vz[:, 0:NB], float(DUMP + 1))
                    idf = dsb.tile([128, CMAX // 128], f32, tag="idf")
                    nc.vector.tensor_tensor(idf[:, 0:NB], idt[:, 0:NB], vz[:, 0:NB],
                                            op=Alu.add)
                    nc.vector.tensor_scalar(idf[:, 0:NB], idf[:, 0:NB], -1.0, None,
                                            op0=Alu.add)
                    nc.vector.tensor_copy(idx_sb[k][:, 0:NB], idf[:, 0:NB])

                # write combine weights (bf16) into xcomb[:, H:H+NSLOT]
                c2bf = dsb.tile([128, T // 128, NSLOT], BF, tag="c2bf")
                nc.vector.tensor_copy(c2bf[:], c2sb[:])
                nc.scalar.dma_start(
                    xcomb.ap()[:, H:H + NSLOT].rearrange("(c p) n -> p c n", p=128),
                    c2bf[:])

            # ====== phase 3: expert MLP per slot ======
            with (
                tc.tile_pool(name="xg", bufs=10) as xgp,
                tc.tile_pool(name="w1", bufs=4) as w1p,
                tc.tile_pool(name="w2", bufs=3) as w2p,
                tc.tile_pool(name="hbuf", bufs=2) as hbp,
                tc.tile_pool(name="xt", bufs=1) as xtp,
                tc.tile_pool(name="ysb", bufs=1) as ysp,
                tc.tile_pool(name="scl", bufs=1) as sclp,
                tc.tile_pool(name="bias", bufs=1) as biasp,
                tc.tile_pool(name="psh", bufs=2, space="PSUM") as psh,
                tc.tile_pool(name="psy", bufs=5, space="PSUM") as psy,
                tc.tile_pool(name="pst", bufs=1, space="PSUM") as pst,
            ):
                b1_sb = biasp.tile([128, NSLOT, F // 128], f32)
                nc.sync.dma_start(b1_sb[:], b1loc.ap().rearrange("l (c p) -> p l c", p=128))
                b2_sb = []
                for kk in range(NSLOT):
                    t = biasp.tile([1, H], BF, tag=f"b2_{kk}", name=f"b2_{kk}")
                    nc.sync.dma_start(t[:], b2loc[kk:kk + 1, :])
                    b2_sb.append(t)

                # phase 3a: hoist ALL slots' row gathers (Pool queue) so no
                # gather parks behind an earlier slot's scatter-adds; the
                # transposes are emitted per-slot in phase 3b so mm1 of slot 0
                # does not queue behind later slots' transposes on the PE
                scls, xgs = [], []
                for k in range(NSLOT):
                    A = caps[k]
                    NB = A // 128
                    scl = sclp.tile([128, CMAX // 128], f32, tag=f"scl{k}",
                                    name=f"scl{k}")
                    scls.append(scl)
                    xgk = []
                    for ck in range(NB):
                        xg = xgp.tile([128, HC], BF, tag="xg")
                        nc.gpsimd.indirect_dma_start(
                            out=xg[:], out_offset=None,
                            in_=xcomb.ap(),
                            in_offset=bass.IndirectOffsetOnAxis(
                                ap=idx_sb[k][:, ck:ck + 1], axis=0),
                            bounds_check=T - 1, oob_is_err=False)
                        nc.vector.tensor_copy(scl[:, ck:ck + 1], xg[:, H + k:H + k + 1])
                        xgk.append(xg)
                    xgs.append(xgk)

                # deferred accumulator zeroing on the otherwise-idle Pool
                # queue: after the gathers, well before the scatter-adds,
                # and off the DMA engines during the router-critical window
                for kk in range(T // 128):
                    for hh in range(2):
                        nc.gpsimd.dma_start(
                            outp[kk * 128:(kk + 1) * 128, hh * HH:(hh + 1) * HH],
                            zero_sb[:])
                nc.gpsimd.dma_start(outp[T:T + 1, 0:HH], zero_sb[0:1, :])
                nc.gpsimd.dma_start(outp[T:T + 1, HH:H], zero_sb[0:1, :])

                for k in range(NSLOT):
                    A = caps[k]
                    NB = A // 128
                    if k == 0 and A > 128:
                        # lead with a 128-token chunk so the first mm1 chain
                        # only waits for the first gather+transpose
                        chs = [128] + ([A - 128] if A - 128 <= 512 else
                                       [512, A - 640])
                        chs = [c for c in chs if c > 0]
                    else:
                        chs = [A] if A <= 512 else [512, A - 512]
                    scl = scls[k]

                    # transposes for this slot only (PE order:
                    # T_k | mm1_k | mm2_k | T_{k+1} ...): 4 per PSUM bank,
                    # one wide strided copy per group
                    xt = xtp.tile([128, H // 128, CMAX], BF, tag=f"xt{k}",
                                  name=f"xt{k}")
                    for ck in range(NB):
                        xg = xgs[k][ck]
                        for g4 in range(H // 512):
                            tp = pst.tile([128, 4, 128], BF, tag="tp3")
                            for j in range(4):
                                hc = g4 * 4 + j
                                nc.tensor.transpose(
                                    tp[:, j, :], xg[:, hc * 128:(hc + 1) * 128],
                                    identb[:])
                            nc.any.tensor_copy(
                                xt[:, g4 * 4:(g4 + 1) * 4, ck * 128:(ck + 1) * 128],
                                tp[:])

                    # mm1: h^T[f, c] = gelu(sum_h W1[h,f]^T x^T[h,c] + b1[f])
                    hbuf = hbp.tile([128, F // 128, CMAX], BF, tag="hbuf")
                    for fo in range(F // FO):
                        w1t = w1p.tile([128, H // 128, FO], BF, tag="w1t")
                        nc.sync.dma_start(
                            w1t[:],
                            W1loc[k, :, fo * FO:(fo + 1) * FO]
                            .rearrange("(c p) f -> p c f", p=128))
                        for fi in range(FO // 128):
                            fg = fo * (FO // 128) + fi
                            cc0 = 0
                            for ch in chs:
                                ph = psh.tile([128, 512], f32, tag="ph")
                                for hc in range(H // 128):
                                    nc.tensor.matmul(
                                        ph[:, 0:ch],
                                        w1t[:, hc, fi * 128:(fi + 1) * 128],
                                        xt[:, hc, cc0:cc0 + ch],
                                        start=(hc == 0), stop=(hc == H // 128 - 1))
                                nc.scalar.activation(
                                    hbuf[:, fg, cc0:cc0 + ch], ph[:, 0:ch],
                                    Act.Gelu, bias=b1_sb[:, k, fg:fg + 1])
                                cc0 += ch

                    # mm2: y[c, h] = (sum_f h^T[f,c]^T W2[f,h] + b2[h]) * s[c]
                    # both column halves are staged into one full-width ysb
                    # per 128-token block, then a single 2KB-row scatter-add
                    ysbs = [ysp.tile([128, H], BF, tag=f"ysb{_i}", name=f"ysb{_i}")
                            for _i in range(NB)]
                    for hh in range(2):
                        pys = [psy.tile([128, HH], f32, tag="py", name=f"py{_i}")
                               for _i in range(NB)]
                        for g in range(F // 512):
                            w2t = w2p.tile([128, 4, HH], BF, tag="w2t")
                            nc.sync.dma_start(
                                w2t[:],
                                W2loc[k, g * 512:(g + 1) * 512, hh * HH:(hh + 1) * HH]
                                .rearrange("(c p) h -> p c h", p=128))
                            for j in range(4):
                                fg = g * 4 + j
                                for ck in range(NB):
                                    nc.tensor.matmul(
                                        pys[ck][:], hbuf[:, fg, ck * 128:(ck + 1) * 128],
                                        w2t[:, j, :], start=(fg == 0), stop=False)
                        for ck in range(NB):
                            nc.tensor.matmul(
                                pys[ck][:], onesmm_sb[0:1, :],
                                b2_sb[k][0:1, hh * HH:(hh + 1) * HH],
                                start=False, stop=True)
                            nc.vector.tensor_scalar(
                                ysbs[ck][:, hh * HH:(hh + 1) * HH], pys[ck][:],
                                scl[:, ck:ck + 1], None, op0=Alu.mult)
                            if hh == 1:
                                nc.gpsimd.indirect_dma_start(
                                    out=outp.ap(),
                                    out_offset=bass.IndirectOffsetOnAxis(
                                        ap=idx_sb[k][:, ck:ck + 1], axis=0),
                                    in_=ysbs[ck][:], in_offset=None,
                                    compute_op=Alu.add,
                                    bounds_check=T, oob_is_err=True)

            # ====== phase 4: single ReduceScatter + output shard ======
            with tc.tile_pool(name="outc", bufs=2) as outc:
                nc.gpsimd.collective_compute(
                    "ReduceScatter", Alu.add, replica_groups=RG,
                    ins=[outp.ap()[0:T, :].opt()], outs=[rsout.ap().opt()])
                for kk in range(TSH // 128):
                    otb = outc.tile([128, H], BF, tag="otb")
                    nc.scalar.dma_start(otb[:], rsout[kk * 128:(kk + 1) * 128, :])
                    ot = outc.tile([128, H], f32, tag="ot")
                    nc.vector.tensor_copy(ot[:], otb[:])
                    nc.scalar.dma_start(out_sh[kk * 128:(kk + 1) * 128, :], ot[:])

    nc.compile()
    if not nc.is_finalized():
        nc.finalize()
    return nc


def _in_maps(inputs, caps, asg):
    import ml_dtypes
    bf16 = ml_dtypes.bfloat16
    NSLOT = len(caps)
    x = np.asarray(inputs["x"], np.float32).reshape(T, H)
    xcomb = np.zeros((T, H + NSLOT), bf16)
    xcomb[:, 0:H] = x.astype(bf16)
    W1 = np.asarray(inputs["W1"], np.float32)
    b1 = np.asarray(inputs["b1"], np.float32)
    W2 = np.asarray(inputs["W2"], np.float32)
    b2 = np.asarray(inputs["b2"], np.float32)
    W1b = W1.astype(bf16)
    W2b = W2.astype(bf16)
    b2b = b2.astype(bf16)
    common = {
        "xcomb": np.ascontiguousarray(xcomb),
        "Wr1": np.ascontiguousarray(np.asarray(inputs["Wr1"], np.float32)),
        "br1": np.ascontiguousarray(np.asarray(inputs["br1"], np.float32)),
        "Wr2": np.ascontiguousarray(np.asarray(inputs["Wr2"], np.float32)),
        "br2": np.ascontiguousarray(np.asarray(inputs["br2"], np.float32)),
    }
    maps = []
    for r in range(NCORES):
        w1l = np.empty((NSLOT, H, F), bf16)
        b1l = np.empty((NSLOT, F), np.float32)
        w2l = np.empty((NSLOT, F, H), bf16)
        b2l = np.empty((NSLOT, H), bf16)
        oh = np.zeros((NSLOT, E), np.float32)
        lo = np.zeros((NSLOT,), np.float32)
        for kk in range(NSLOT):
            e, l0 = asg[kk][r]
            w1l[kk] = W1b[e]
            b1l[kk] = b1[e]
            w2l[kk] = W2b[e]
            b2l[kk] = b2b[e]
            if l0 <= T:
                oh[kk, e] = 1.0     # empty slots keep an all-zero one-hot
            lo[kk] = float(l0)
        maps.append({
            **common,
            "xsh": np.ascontiguousarray(x[r * TSH:(r + 1) * TSH]),
            "W1loc": w1l, "b1loc": b1l, "W2loc": w2l, "b2loc": b2l,
            "ohloc": oh, "slotlo": lo,
        })
    return maps


def _get_nc(caps):
    key = tuple(caps)
    if key not in _CACHE:
        _CACHE[key] = _build(list(caps))
    return _CACHE[key]


def kernel(**inputs) -> np.ndarray:
    from concourse.bass_utils import run_bass_kernel_spmd

    caps, asg = _plan(inputs)
    nc = _get_nc(caps)
    maps = _in_maps(inputs, caps, asg)
    res = run_bass_kernel_spmd(nc, maps, core_ids=list(range(NCORES)))
    shards = [res.results[r]["out_sh"] for r in range(NCORES)]
    out = np.concatenate(shards, axis=0).reshape(np.asarray(inputs["x"]).shape)
    return out.astype(np.float32)
